# revision 1
# baseline (speedup 1.0000x reference)
"""Multi-head attention (B=4, S=2048, d_model=1024, 16 heads x 64) on 8 trn2
NeuronCores.

Sharding: core c -> (batch b = c//2, head-group g = c%2). Each core computes
batch b restricted to heads [g*8, g*8+8): QKV projections for those 512 of the
1024 qkv dims, attention for 8 heads, and the partial output projection
out_partial = attn_out_cat @ Wo[g*512:(g+1)*512, :].  Host sums the two
partials per batch and adds bo.

Device-side layout: everything transposed ("d on partitions, seq on free").
  qhT/khT pair tiles [128 = 2 heads x 64, 2048] (bf16)
  scoresT[k, q] via row-tiled head-pair matmuls (K=64 each, concurrent)
  PT = exp(scoresT/8) on ScalarE directly from PSUM
  AV: out_T[65, q] = [vh | ones]^T @ PT  (row 64 = softmax denominators)
  normalize with vector reciprocal + DMA partition-broadcast via DRAM
  final: out[q, 1024] = outTn^T @ Wo_slice
Matmuls run as float32r (1 cycle/row) or bf16; plain fp32 is 4 cycles/row.
"""

import numpy as np

import concourse.bass as bass
import concourse.bacc as bacc
import concourse.mybir as mybir
import concourse.tile as tile
from concourse import bass_utils

F32 = mybir.dt.float32
R32 = mybir.dt.float32r
BF16 = mybir.dt.bfloat16

B, S, DM = 4, 2048, 1024
HPC = 8          # heads per core
DK = DV = 64
NP = HPC // 2    # head pairs per core = 4
KT = S // 128    # 16 k-tiles
KC = DM // 128   # 8 contraction chunks


def build_nc(debug_taps=False):
    nc = bacc.Bacc("TRN2", target_bir_lowering=False)
    taps = {}
    if debug_taps:
        taps["qhT0"] = nc.dram_tensor("tap_qhT0", [128, S], F32,
                                      kind="ExternalOutput")
        taps["av"] = nc.dram_tensor("tap_av", [DV + 1, 512], F32,
                                    kind="ExternalOutput")
        taps["outTn0"] = nc.dram_tensor("tap_outTn0", [128, S], F32,
                                        kind="ExternalOutput")

    # host-pre-tiled inputs: x[kc, qq, 128, 512] contiguous chunks; weights
    # already in on-chip layout
    qT = nc.dram_tensor("qT", [4, KC, 128, 512], R32, kind="ExternalInput")
    kT = nc.dram_tensor("kT", [4, KC, 128, 512], R32, kind="ExternalInput")
    vT = nc.dram_tensor("vT", [KC, 4, 128, 512], R32, kind="ExternalInput")
    wq = nc.dram_tensor("wq", [128, KC, 512], R32, kind="ExternalInput")
    wk = nc.dram_tensor("wk", [128, KC, 512], R32, kind="ExternalInput")
    wv = nc.dram_tensor("wv", [128, KC, 512], R32, kind="ExternalInput")
    wo = nc.dram_tensor("wo", [128, 4, 1024], R32, kind="ExternalInput")
    bq = nc.dram_tensor("bq", [128, NP], F32, kind="ExternalInput")
    bk = nc.dram_tensor("bk", [128, NP], F32, kind="ExternalInput")
    bv = nc.dram_tensor("bv", [512], F32, kind="ExternalInput")
    out = nc.dram_tensor("out", [S, 1024], F32, kind="ExternalOutput")
    # scratch for softmax-denominator reciprocal partition-broadcast
    rscr = nc.dram_tensor("rscr", [NP * 4 * 2, 512], F32, kind="Internal")

    with tile.TileContext(nc) as tc:
        from contextlib import ExitStack

        with ExitStack() as est:
            # ---------------- persistent SBUF pools ----------------
            wqk_pool = est.enter_context(tc.tile_pool(name="wqk", bufs=1))
            wo_pool = est.enter_context(tc.tile_pool(name="wop", bufs=1))
            bias_pool = est.enter_context(tc.tile_pool(name="bias", bufs=1))
            vh_pool = est.enter_context(tc.tile_pool(name="vhp", bufs=1))
            qkt_pool = est.enter_context(tc.tile_pool(name="qkt", bufs=1))
            x_pool = est.enter_context(tc.tile_pool(name="xch", bufs=3))

            wq_sb = wqk_pool.tile([128, KC, 512], R32, name="wq_sb")
            wk_sb = wqk_pool.tile([128, KC, 512], R32, name="wk_sb")
            wo_sb = wo_pool.tile([128, 4, 1024], R32, name="wo_sb")
            bq_sb = bias_pool.tile([128, NP], F32, name="bq_sb")
            bk_sb = bias_pool.tile([128, NP], F32, name="bk_sb")
            bv_ap = bv[:]
            bv_bc = bias_pool.tile([128, 512], F32, name="bv_bc")
            nc.sync.dma_start(
                out=bv_bc,
                in_=bass.AP(tensor=bv_ap.tensor, offset=bv_ap.offset,
                            ap=[[0, 128]] + list(bv_ap.ap)),
            )

            # vh_all[:, h, t, 0:64] = vh rows t*128..t*128+128 for head h
            # vh_all[:, h, t, 64] = 1.0 (denominator column)
            vh_all = vh_pool.tile([128, HPC, KT, DV + 1], R32, name="vh_all")
            nc.vector.memset(vh_all[:, :, :, DV:DV + 1].bitcast(F32), 1.0)

            # ---------------- phase V: v projection (all 8 heads) ----------
            with tc.tile_pool(name="wvp", bufs=1) as wv_pool, \
                 tc.tile_pool(name="psV", bufs=8, space="PSUM") as psV:
                wv_sb = wv_pool.tile([128, KC, 512], R32, name="wv_sb")
                nc.sync.dma_start(out=wv_sb, in_=wv[:, :, :])
                for sq in range(4):           # S quarters
                    pss = []
                    for j in range(4):
                        ps = psV.tile([128, 512], F32, name=f"psv{j}",
                                      tag="psv")
                        pss.append(ps)
                    for kc in range(KC):
                        vch = x_pool.tile([128, 512], R32, name="vch",
                                          tag="xv", bufs=6)
                        nc.scalar.dma_start(out=vch, in_=vT[kc, sq])
                        for j in range(4):
                            nc.tensor.matmul(
                                pss[j],
                                lhsT=vch[:, j * 128:(j + 1) * 128],
                                rhs=wv_sb[:, kc, :],
                                start=(kc == 0), stop=(kc == KC - 1))
                    for j in range(4):
                        st = sq * 4 + j
                        nc.vector.tensor_add(
                            vh_all[:, :, st, 0:DV],
                            pss[j].rearrange("p (h d) -> p h d", h=HPC),
                            bv_bc.rearrange("p (h d) -> p h d", h=HPC))

            # weights for A/C load during phase V compute
            nc.sync.dma_start(out=wq_sb, in_=wq[:, :, :])
            nc.sync.dma_start(out=wk_sb, in_=wk[:, :, :])
            nc.sync.dma_start(out=wo_sb, in_=wo[:, :, :])
            nc.sync.dma_start(out=bq_sb, in_=bq[:, :])
            nc.sync.dma_start(out=bk_sb, in_=bk[:, :])

            # ---------------- phase A: q/k projections, all pairs ----------
            qhTs, khTs, outTns = {}, {}, {}
            for p in range(NP):
                qhTs[p] = qkt_pool.tile([128, S], BF16, name=f"qhT{p}",
                                        tag=f"qhT{p}")
                khTs[p] = qkt_pool.tile([128, S], BF16, name=f"khT{p}",
                                        tag=f"khT{p}")
            with tc.tile_pool(name="psA", bufs=1, space="PSUM") as psA:
                for qq in range(4):
                    psq = [psA.tile([128, 512], F32, name=f"psq{p}",
                                    tag=f"paq{p}") for p in range(NP)]
                    psk = [psA.tile([128, 512], F32, name=f"psk{p}",
                                    tag=f"pak{p}") for p in range(NP)]
                    for kc in range(KC):
                        qch = x_pool.tile([128, 512], R32, name="qch",
                                          tag="xq", bufs=4)
                        nc.sync.dma_start(out=qch, in_=qT[qq, kc])
                        kch = x_pool.tile([128, 512], R32, name="kch",
                                          tag="xk", bufs=4)
                        nc.scalar.dma_start(out=kch, in_=kT[qq, kc])
                        for p in range(NP):
                            nc.tensor.matmul(
                                psq[p],
                                lhsT=wq_sb[:, kc, p * 128:(p + 1) * 128],
                                rhs=qch,
                                start=(kc == 0), stop=(kc == KC - 1))
                            nc.tensor.matmul(
                                psk[p],
                                lhsT=wk_sb[:, kc, p * 128:(p + 1) * 128],
                                rhs=kch,
                                start=(kc == 0), stop=(kc == KC - 1))
                    sl = slice(qq * 512, (qq + 1) * 512)
                    for p in range(NP):
                        nc.vector.tensor_scalar_add(qhTs[p][:, sl], psq[p],
                                                    bq_sb[:, p:p + 1])
                        nc.vector.tensor_scalar_add(khTs[p][:, sl], psk[p],
                                                    bk_sb[:, p:p + 1])
            if debug_taps:
                nc.gpsimd.dma_start(out=taps["qhT0"][:, :], in_=qhTs[0])

            # ---------------- phase B: attention ----------------
            outtn_pool = est.enter_context(tc.tile_pool(name="otn", bufs=1))
            pt_pool = est.enter_context(tc.tile_pool(name="ptp", bufs=2))
            rc_pool = est.enter_context(tc.tile_pool(name="rcp", bufs=2))
            bc_pool = est.enter_context(tc.tile_pool(name="bcp", bufs=2))
            fin_pool = est.enter_context(tc.tile_pool(name="finp", bufs=2))

            with tc.tile_pool(name="psS", bufs=2, space="PSUM") as psS, \
                 tc.tile_pool(name="psAV", bufs=1, space="PSUM") as psAV, \
                 tc.tile_pool(name="psC", bufs=2, space="PSUM") as psC:
                for p in range(NP):
                    outTns[p] = outtn_pool.tile([128, S], R32,
                                                name=f"outTn{p}",
                                                tag=f"otn{p}")
                for qc in range(4):
                    qsl = slice(qc * 512, (qc + 1) * 512)
                    for p in range(NP):
                        qhT_p, khT_p = qhTs[p], khTs[p]
                        avs = [psAV.tile([DV + 1, 512], F32, name=f"av{i}",
                                         tag=f"av{i}") for i in (0, 1)]
                        for kt in range(KT):
                            sc = psS.tile([128, 1024], F32, name="sc",
                                          tag="sc")
                            ksl = slice(kt * 128, (kt + 1) * 128)
                            nc.tensor.matmul(sc[:, 0:512],
                                             lhsT=khT_p[0:64, ksl],
                                             rhs=qhT_p[0:64, qsl],
                                             start=True, stop=True)
                            nc.tensor.matmul(sc[:, 512:1024],
                                             lhsT=khT_p[64:128, ksl],
                                             rhs=qhT_p[64:128, qsl],
                                             start=True, stop=True)
                            pt = pt_pool.tile([128, 1024], R32, name="pt",
                                              tag="pt")
                            nc.scalar.activation(
                                pt, sc, mybir.ActivationFunctionType.Exp,
                                scale=0.125)
                            for i in (0, 1):
                                nc.tensor.matmul(
                                    avs[i],
                                    lhsT=vh_all[:, 2 * p + i, kt, :]
                                    ,
                                    rhs=pt[:, i * 512:(i + 1) * 512]
                                    ,
                                    start=(kt == 0), stop=(kt == KT - 1))
                        # evict AV psum immediately (frees the banks), then
                        # normalize off the critical path
                        av_sb = fin_pool.tile([DV + 1, 2, 512], F32,
                                              name="av_sb", tag="avsb")
                        for i in (0, 1):
                            nc.vector.tensor_copy(av_sb[:, i, :], avs[i])
                        if debug_taps and p == 0 and qc == 0:
                            nc.sync.dma_start(out=taps["av"][:, :],
                                              in_=av_sb[:, 0, :])
                        for i in (0, 1):
                            rc = rc_pool.tile([1, 512], F32, name="rc",
                                              tag="rc")
                            nc.vector.reciprocal(rc, av_sb[DV:DV + 1, i, :])
                            slot = (p * 4 + qc) * 2 + i
                            nc.sync.dma_start(out=rscr[slot:slot + 1, :],
                                              in_=rc)
                            bc = bc_pool.tile([64, 512], F32, name="bc",
                                              tag="bc")
                            nc.sync.dma_start(
                                out=bc,
                                in_=bass.AP(tensor=rscr[:].tensor,
                                            offset=slot * 512,
                                            ap=[[0, 64], [1, 512]]))
                            nc.vector.tensor_mul(
                                outTns[p][i * 64:(i + 1) * 64, qsl],
                                av_sb[0:DV, i, :], bc)
                    # ---- output projection, delayed one qc so the next
                    # chunk's scores outrank it on PE (keeps ACT fed) ----
                    for qcd in ([qc - 1] if qc > 0 else []) + (
                            [qc] if qc == 3 else []):
                        for j in range(4):
                            qt = qcd * 4 + j
                            tsl = slice(qt * 128, (qt + 1) * 128)
                            pf0 = psC.tile([128, 512], F32, name="pf0",
                                           tag="fin")
                            pf1 = psC.tile([128, 512], F32, name="pf1",
                                           tag="fin")
                            for c in range(4):
                                lhsT = outTns[c][:, tsl]
                                nc.tensor.matmul(
                                    pf0, lhsT=lhsT,
                                    rhs=wo_sb[:, c, 0:512],
                                    start=(c == 0), stop=(c == 3))
                                nc.tensor.matmul(
                                    pf1, lhsT=lhsT,
                                    rhs=wo_sb[:, c, 512:1024],
                                    start=(c == 0), stop=(c == 3))
                            fs = fin_pool.tile([128, 1024], F32, name="fs",
                                               tag="fs")
                            nc.vector.tensor_copy(fs[:, 0:512], pf0)
                            nc.vector.tensor_copy(fs[:, 512:1024], pf1)
                            nc.sync.dma_start(out=out[tsl, :], in_=fs)
                if debug_taps:
                    nc.sync.dma_start(out=taps["outTn0"][:, :],
                                      in_=outTns[0].bitcast(F32))

    nc.compile()
    return nc


_NC = None


def _get_nc():
    global _NC
    if _NC is None:
        _NC = build_nc()
    return _NC


def make_in_maps(inputs):
    q = np.asarray(inputs["q"], dtype=np.float32)
    k = np.asarray(inputs["k"], dtype=np.float32)
    v = np.asarray(inputs["v"], dtype=np.float32)
    Wq = np.asarray(inputs["Wq"], dtype=np.float32)
    Wk = np.asarray(inputs["Wk"], dtype=np.float32)
    Wv = np.asarray(inputs["Wv"], dtype=np.float32)
    Wo = np.asarray(inputs["Wo"], dtype=np.float32)
    bq = np.asarray(inputs["bq"], dtype=np.float32)
    bk = np.asarray(inputs["bk"], dtype=np.float32)
    bv = np.asarray(inputs["bv"], dtype=np.float32)

    in_maps = []
    for c in range(8):
        b, g = divmod(c, 2)
        sl = slice(g * 512, (g + 1) * 512)

        def tile_xT(x, qq_major):
            # x[b].T [1024, 2048] -> chunks [128, 512]; qq_major picks
            # [qq, kc, 128, 512] vs [kc, qq, 128, 512]
            xt = x.T.reshape(KC, 128, 4, 512)
            perm = (2, 0, 1, 3) if qq_major else (0, 2, 1, 3)
            return np.ascontiguousarray(xt.transpose(*perm))

        def tile_w(W):
            # [1024, 512] -> [128, kc, 512]
            return np.ascontiguousarray(
                W[:, sl].reshape(KC, 128, 512).transpose(1, 0, 2))

        in_maps.append({
            "qT": tile_xT(q[b], True),
            "kT": tile_xT(k[b], True),
            "vT": tile_xT(v[b], False),
            "wq": tile_w(Wq),
            "wk": tile_w(Wk),
            "wv": tile_w(Wv),
            "wo": np.ascontiguousarray(
                Wo[sl, :].reshape(4, 128, 1024).transpose(1, 0, 2)),
            "bq": np.ascontiguousarray(bq[sl].reshape(NP, 128).T),
            "bk": np.ascontiguousarray(bk[sl].reshape(NP, 128).T),
            "bv": np.ascontiguousarray(bv[sl]),
        })
    return in_maps


def gather_output(results, inputs):
    bo = np.asarray(inputs["bo"], dtype=np.float32)
    outs = [np.asarray(r["out"]) for r in results]
    full = np.stack([outs[2 * b] + outs[2 * b + 1] + bo for b in range(B)])
    return full.astype(np.float32)


def kernel(**inputs):
    nc = _get_nc()
    in_maps = make_in_maps(inputs)
    res = bass_utils.run_bass_kernel_spmd(nc, in_maps, core_ids=list(range(8)))
    return gather_output(res.results, inputs)


if __name__ == "__main__":
    build_nc()
    print("build OK")

